# revision 43
# baseline (speedup 1.0000x reference)
"""Trainium2 Bass kernel for nn_Conv_agg_raw (GNN message passing).

Math: out = sum_k weight[k] @ (h @ resx[k]) + bias, where resx[k] is the
dense [N,N] scatter of edge features X[:,k] at (src,dst).  Equivalently
    res_k[:, m] = sum_{e: dst_e=m} X[e,k] * h[:, src_e]
    out[:, m]   = sum_k weight[k] @ res_k[:, m] + bias
We shard dst nodes across the 8 cores (512 each) - fully independent, no
collectives.  Each core gathers h columns by src (DMA gather of h^T rows),
aggregates edges into res via PE matmuls against on-the-fly built
(x outer dst-one-hot) matrices, then applies the stacked weight.
"""

import time
import numpy as np

import concourse.bass as bass
import concourse.bacc as bacc
import concourse.tile as tile
from concourse import mybir
from concourse.bass_utils import run_bass_kernel_spmd

import os as _os

N = 4096
K = 8
C = 256
NCORES = 8
DSTS_PER_CORE = N // NCORES      # 512
SLOTS = int(_os.environ.get("GNN_SLOTS", "16"))  # dst slots per window
WINDOWS = DSTS_PER_CORE // SLOTS                 # windows per core

_prog_cache: dict = {}


# ---------------------------------------------------------------- device ----
def _build_program(caps, use_f32r: bool = True, gather_group: int = 2,
                   hg_bufs: int = 6, rhs_bufs: int = 6, gp_off: int = 1,
                   copy_dve8: int = 1, final_split: int = 2):
    """Build the per-core Bass program.

    caps[w] = number of 128-edge chunks the w-th window holds (same vector
    for every core; the host packs each core's dsts to fit it)."""
    caps = list(caps)
    assert len(caps) == WINDOWS
    off = [0]
    for cw in caps:
        off.append(off[-1] + cw)
    nchunk = off[-1]                 # total chunks per core

    nc = bacc.Bacc("TRN2", target_bir_lowering=False, debug=False)
    f32 = mybir.dt.float32
    f32r = mybir.dt.float32r if use_f32r else mybir.dt.float32
    # h rows travel as fp16 (halves gather DMA); f32r already rounds the PE
    # inputs to ~13 mantissa bits, so fp16 h costs little extra accuracy.
    hdt = mybir.dt.float16 if use_f32r else mybir.dt.float32

    hT = nc.dram_tensor("hT", [N, C], hdt, kind="ExternalInput")
    wT = nc.dram_tensor("wT", [K * C, C], f32r, kind="ExternalInput")
    bias_d = nc.dram_tensor("bias", [2, 128], f32, kind="ExternalInput")
    idx_d = nc.dram_tensor("idx", [128, nchunk * 8], mybir.dt.int16,
                           kind="ExternalInput")
    xr_d = nc.dram_tensor("xr", [128, nchunk * K], f32, kind="ExternalInput")
    dl_d = nc.dram_tensor("dl", [128, nchunk], f32, kind="ExternalInput")
    iota_d = nc.dram_tensor("iota", [128, SLOTS], f32, kind="ExternalInput")
    out_d = nc.dram_tensor("out", [C, DSTS_PER_CORE], f32,
                           kind="ExternalOutput")

    GG = gather_group
    assert WINDOWS % GG == 0

    with tile.TileContext(nc) as tc:
        with (
            tc.tile_pool(name="persist", bufs=1) as pp,
            tc.tile_pool(name="hg", bufs=hg_bufs) as hgp,
            tc.tile_pool(name="rhs", bufs=rhs_bufs) as rhp,
            tc.tile_pool(name="outp", bufs=2) as op,
            tc.tile_pool(name="psw", bufs=3, space="PSUM") as psw,
            tc.tile_pool(name="psf", bufs=2, space="PSUM") as psf,
        ):
            # ---- bulk loads (idx first: it gates the first gather) ----
            idx_sb = pp.tile([128, nchunk * 8], mybir.dt.int16)
            nc.sync.dma_start(idx_sb[:], idx_d.ap())
            dl_sb = pp.tile([128, nchunk], f32)
            nc.sync.dma_start(dl_sb[:], dl_d.ap())
            iota_sb = pp.tile([128, SLOTS], f32)
            nc.sync.dma_start(iota_sb[:], iota_d.ap())
            xr_sb = pp.tile([128, nchunk * K], f32)
            nc.sync.dma_start(xr_sb[:], xr_d.ap())
            bias_sb = pp.tile([128, 2], f32)
            # bias_d[half] -> bias_sb[:, half]
            nc.sync.dma_start(
                bias_sb[:],
                bass.AP(bias_d, 0, [[1, 128], [128, 2]]),
            )
            # ---- dst-slot one-hot mask for every chunk: one big DVE op ----
            # mask_all[p, ch, d] = (dl[p, ch] == d)
            mask_all = pp.tile([128, nchunk * SLOTS], f32)
            nc.vector.tensor_tensor(
                mask_all[:].rearrange("p (ch d) -> p ch d", d=SLOTS),
                dl_sb[:].unsqueeze(-1).broadcast_to([128, nchunk, SLOTS]),
                iota_sb[:].unsqueeze(1).broadcast_to([128, nchunk, SLOTS]),
                mybir.AluOpType.is_equal,
            )

            resstack = [pp.tile([128, WINDOWS * K * SLOTS], f32r,
                                name=f"resstack{i}")
                        for i in range(2)]   # per c_in half; col = w*256+k*32+d

            # wT feeds only the final matmuls; emit its load mid-stream so it
            # does not delay the first gathers.
            wt_sb = pp.tile([128, 16 * C], f32r)  # chunk q at cols [q*256,..)
            wt_loaded = False

            for g in range(WINDOWS // GG):
                if g * GG >= WINDOWS // 2 and not wt_loaded:
                    nc.sync.dma_start(
                        wt_sb[:], wT.ap().rearrange("(q p) c -> p q c", p=128))
                    wt_loaded = True
                # gather h^T rows for this window group's (padded) edge list
                w0 = g * GG
                gch = off[w0 + GG] - off[w0]   # chunks in this group
                hg = hgp.tile([128, gch, C], hdt, tag="hg")
                nc.gpsimd.dma_gather(
                    out_ap=hg[:],
                    in_ap=hT.ap(),
                    idxs_ap=idx_sb[:, off[w0] * 8:off[w0 + GG] * 8],
                    num_idxs=gch * 128,
                    num_idxs_reg=gch * 128,
                    elem_size=C,
                    # single-packet mode traps the exec unit above ~1024 descs
                    single_packet=False,
                )
                for wl in range(GG):
                    w = w0 + wl
                    cw = caps[w]
                    # rhs[p, ch, k, d] = xr[p, ch, k] * mask[p, ch, d]
                    rhs = rhp.tile([128, cw, K * SLOTS], hdt, tag="rhs")
                    xr_w = bass.AP(xr_sb[:].tensor,
                                   xr_sb[:].offset + off[w] * K,
                                   [[nchunk * K, 128], [K, cw],
                                    [1, K], [0, SLOTS]])
                    mk_w = bass.AP(mask_all[:].tensor,
                                   mask_all[:].offset + off[w] * SLOTS,
                                   [[nchunk * SLOTS, 128], [SLOTS, cw],
                                    [0, K], [1, SLOTS]])
                    build_eng = (nc.gpsimd if (w % WINDOWS) < gp_off
                                 else nc.vector)
                    build_eng.tensor_tensor(
                        rhs[:].rearrange("p ch (k d) -> p ch k d", d=SLOTS),
                        xr_w, mk_w, mybir.AluOpType.mult,
                    )

                    # aggregate: psum[ci, (k,d)] += Hg_chunk.T @ rhs_chunk
                    ps = [psw.tile([128, K * SLOTS], f32, tag=f"psw{half}",
                                   name=f"ps{half}") for half in range(2)]
                    for ch in range(cw):
                        for half in range(2):
                            nc.tensor.matmul(
                                ps[half][:],
                                hg[:, off[w] - off[w0] + ch,
                                   half * 128:(half + 1) * 128],
                                rhs[:, ch, :],
                                start=(ch == 0),
                                stop=(ch == cw - 1),
                            )
                    for half in range(2):
                        dst_slice = resstack[half][:, w * K * SLOTS:
                                                   (w + 1) * K * SLOTS]
                        if (2 * w + half) % 8 < copy_dve8:
                            nc.vector.tensor_copy(dst_slice, ps[half][:])
                        else:
                            nc.scalar.copy(dst_slice, ps[half][:])

            # ---- apply stacked weight: out[co, m] = sum_{k,ci} wT·res ----
            # Split over window halves so the first half's matmuls overlap
            # the second half of the edge stream.
            HW2 = WINDOWS // final_split
            M2 = HW2 * SLOTS
            for fh in range(final_split):
                for oh in range(2):
                    pso = psf.tile([128, M2], f32, tag="psf")
                    for q in range(16):      # q = (k, ci_half)
                        k, cih = divmod(q, 2)
                        rs = resstack[cih][:]
                        rhs_ap = bass.AP(
                            rs.tensor,
                            rs.offset + fh * HW2 * K * SLOTS + k * SLOTS,
                            [[WINDOWS * K * SLOTS, 128],
                             [K * SLOTS, HW2], [1, SLOTS]],
                        )
                        nc.tensor.matmul(
                            pso[:],
                            wt_sb[:, q * 256 + oh * 128:
                                  q * 256 + oh * 128 + 128],
                            rhs_ap,
                            start=(q == 0),
                            stop=(q == 15),
                        )
                    out_sb = op.tile([128, M2], f32, tag="osb")
                    nc.scalar.add(out_sb[:], pso[:], bias_sb[:, oh:oh + 1])
                    nc.sync.dma_start(
                        out_d.ap()[oh * 128:(oh + 1) * 128,
                                   fh * M2:(fh + 1) * M2],
                        out_sb[:])

    nc.compile()
    return nc


# ------------------------------------------------------------------ host ----
def _greedy_partition(items_deg, nbins, cap):
    """Assign item ids (sorted desc by degree) to bins; each bin gets at most
    `cap` items, minimizing max degree-sum.  Returns list of lists."""
    import heapq
    bins = [[] for _ in range(nbins)]
    heap = [(0, b) for b in range(nbins)]
    heapq.heapify(heap)
    for it, dg in items_deg:
        s, b = heapq.heappop(heap)
        bins[b].append(it)
        if len(bins[b]) < cap:
            heapq.heappush(heap, (s + dg, b))
    return bins


def _pack_windows(items_deg, caps):
    """Pack (dst, deg) items into len(caps) bins of SLOTS items each with
    bin w's degree-sum <= caps[w]*128.  Returns list of lists or None."""
    nb = len(caps)
    rem_cap = [c * 128 for c in caps]
    rem_slots = [SLOTS] * nb
    bins = [[] for _ in range(nb)]
    for it, dg in items_deg:           # desc by degree
        # feasible bins; choose max remaining cap per remaining slot
        best, best_score = -1, None
        for b in range(nb):
            if rem_slots[b] == 0 or rem_cap[b] < dg:
                continue
            score = (rem_cap[b] - dg) / rem_slots[b]
            if best_score is None or score > best_score:
                best, best_score = b, score
        if best < 0:
            return None
        bins[best].append(it)
        rem_cap[best] -= dg
        rem_slots[best] -= 1
    return bins


def kernel(h, X, edge_index, batch_node, weight, bias):
    h = np.asarray(h, dtype=np.float32)
    X = np.asarray(X, dtype=np.float32)
    edge_index = np.asarray(edge_index)
    weight = np.asarray(weight, dtype=np.float32)
    bias = np.asarray(bias, dtype=np.float32)

    src = edge_index[0].astype(np.int64)
    dst = edge_index[1].astype(np.int64)
    E = src.shape[0]

    deg = np.bincount(dst, minlength=N)
    order = np.argsort(-deg, kind="stable")

    # dst -> core (8 bins of 512), then per core dst -> window (16 bins of 32)
    core_bins = _greedy_partition([(int(m), int(deg[m])) for m in order],
                                  NCORES, DSTS_PER_CORE)

    # edges grouped by dst
    eorder = np.argsort(dst, kind="stable")
    starts = np.searchsorted(dst[eorder], np.arange(N))
    ends = np.searchsorted(dst[eorder], np.arange(N) + 1)

    # Per-window-ordinal chunk capacities (shared by all cores): a few wide
    # windows absorb the degree-sum variance so most can stay tight.
    base = -(-E // (NCORES * WINDOWS * 128))       # avg chunks per window
    core_windows = None
    caps = None
    for nwide in (WINDOWS // 4, WINDOWS // 2, WINDOWS):
        caps_try = [base + 1] * nwide + [base] * (WINDOWS - nwide)
        packs = []
        for c in range(NCORES):
            items = [(m, int(deg[m])) for m in
                     sorted(core_bins[c], key=lambda m: -deg[m])]
            p = _pack_windows(items, caps_try)
            if p is None:
                break
            packs.append(p)
        if len(packs) == NCORES:
            core_windows, caps = packs, caps_try
            break
    if core_windows is None:
        # fallback: uniform capacity from the worst window under plain LPT
        max_cnt = 0
        core_windows = []
        for c in range(NCORES):
            items = [(m, int(deg[m])) for m in
                     sorted(core_bins[c], key=lambda m: -deg[m])]
            wins = _greedy_partition(items, WINDOWS, SLOTS)
            core_windows.append(wins)
            for wlist in wins:
                max_cnt = max(max_cnt, int(sum(deg[m] for m in wlist)))
        caps = [max(1, -(-max_cnt // 128))] * WINDOWS

    import os
    use_f32r = os.environ.get("GNN_MM_MODE", "f32r") != "fp32"
    gg = 2 if WINDOWS >= 32 else 1
    key = (tuple(caps), use_f32r, SLOTS)
    if key not in _prog_cache:
        _prog_cache[key] = _build_program(caps, use_f32r=use_f32r,
                                          gather_group=gg)
    nc = _prog_cache[key]

    off = [0]
    for cw in caps:
        off.append(off[-1] + cw)
    nchunk = off[-1]
    hT = np.ascontiguousarray(h.T)                            # [N, C]
    if use_f32r:
        hT = hT.astype(np.float16)
    wT = np.ascontiguousarray(
        weight.transpose(0, 2, 1).reshape(K * C, C))          # [(k,ci), co]
    bias2 = np.ascontiguousarray(bias.reshape(2, 128))
    iota = np.broadcast_to(np.arange(SLOTS, dtype=np.float32),
                           (128, SLOTS)).copy()

    in_maps = []
    perms = []
    for c in range(NCORES):
        idx = np.zeros((128, nchunk * 8), dtype=np.int16)
        xr = np.zeros((128, nchunk, K), dtype=np.float32)
        dl = np.zeros((128, nchunk), dtype=np.float32)
        perm = np.empty(DSTS_PER_CORE, dtype=np.int64)
        for w in range(WINDOWS):
            wl = core_windows[c][w]
            el = []
            sl = []
            for d_slot, m in enumerate(wl):
                perm[w * SLOTS + d_slot] = m
                ee = eorder[starts[m]:ends[m]]
                el.append(ee)
                sl.append(np.full(ee.shape[0], d_slot, dtype=np.float32))
            el = (np.concatenate(el) if el else
                  np.empty(0, dtype=np.int64))
            sl = (np.concatenate(sl) if sl else
                  np.empty(0, dtype=np.float32))
            # order the window's edges by src: the dst slot travels in `sl`,
            # and src-sorted gathers hit HBM row buffers far more often
            so = np.argsort(src[el], kind="stable")
            el, sl = el[so], sl[so]
            L = el.shape[0]
            j = np.arange(L)
            p = j % 128
            ch = off[w] + j // 128
            xr[p, ch, :] = X[el, :]
            dl[p, ch] = sl
            # gather index layout: pos j -> [j%16, j//16], replicated x8
            srcs = src[el].astype(np.int16)
            blk = np.zeros((16, caps[w] * 8), dtype=np.int16)
            blk[j % 16, j // 16] = srcs
            idx[:, off[w] * 8:off[w + 1] * 8] = np.tile(blk, (8, 1))
        in_maps.append({
            "hT": hT, "wT": wT, "bias": bias2, "iota": iota,
            "idx": idx,
            "xr": np.ascontiguousarray(xr.reshape(128, nchunk * K)),
            "dl": dl,
        })
        perms.append(perm)

    global _last_perms
    _last_perms = perms

    try:
        res = run_bass_kernel_spmd(nc, in_maps, core_ids=list(range(NCORES)))
    except Exception:
        # transient device-state issues (e.g. a previous crashed process left
        # a core unrecoverable) usually clear on retry
        time.sleep(10)
        res = run_bass_kernel_spmd(nc, in_maps, core_ids=list(range(NCORES)))

    out = np.empty((C, N), dtype=np.float32)
    for c in range(NCORES):
        out[:, perms[c]] = res.results[c]["out"]
    return out
